# revision 1
# baseline (speedup 1.0000x reference)
"""Trainium2 Bass kernel for nn_AST_LSTM (GRU-based AST message passing).

Algorithm notes
---------------
The reference peels leaf edges of a random tree for 15 iterations; the
edge schedule (which edges fire when, and the compacted index remapping)
depends ONLY on E, so it is precomputed on the host. Per iteration the
device does, for each of 8 row-sharded cores:

    q = S_k @ h            (sparse mean-aggregate of gathered rows)
    G = [q | h] @ Wcat + b  with Wcat = [conv_w @ w_ih.T ; w_hh.T]
    r = sigmoid(G[:, :384]); z = sigmoid(G[:, 384:768])
    n = tanh(gi_n + b_ih_n + r * (gh_n + b_hh_n))
    h' = n + z * (h - n)

Rows are block-cyclically sharded (block=384) over 8 cores; each core
holds h transposed in SBUF as 10 window tiles [128, 3, 384] (feature
partition-major) so all matmuls run directly.  Message sources are
exchanged once per iteration with an AllGather of just the needed rows
(the "halo"); iteration 0 gathers from a full local copy of V.

Precision: the r/z gate matmuls run in fp8e4m3 (weights pre-scaled by
16, the sigmoid un-scales via its scale port) using DoubleRow perf mode
for the first two K-blocks; the n-gate matmuls stay bf16 (its error
passes through tanh undamped).  h lives in SBUF as bf16 plus an fp8
shadow copy refreshed each iteration on the Pool engine.  PSUM
accumulates fp32.  r/z biases ride the sigmoid bias port; b_hh_n is
fused into the DVE (gh_n + b) * r op; b_ih_n rides the tanh bias port,
so no bias matmuls remain.  Each window's tanh/h' tail is software-
pipelined one window behind its matmuls, cold windows run first each
iteration to cover the collective, and the AllGather fires as soon as
the last source window's h' lands.
"""
import os
import sys
import numpy as np

sys.path.insert(0, "/opt/trn_rl_repo")
import ml_dtypes

N = 30000
D = 384
NC = 8
ITERS = int(os.environ.get("KERNEL_ITERS", "15"))
B = 384              # assignment block == window width
WPC = 10             # windows per core
LROWS = B * WPC      # 3840 local rows (padded)
NBLKS = (N + B - 1) // B
OOB = 1 << 20
BF16 = ml_dtypes.bfloat16
FP8 = ml_dtypes.float8_e4m3
W8SCALE = 16.0

# ----------------------------------------------------------------------------
# host-side schedule
# ----------------------------------------------------------------------------

def _local_row(g):
    return ((g // B) // NC) * B + g % B


def _global_rows_of(c):
    out = np.full(LROWS, -1, dtype=np.int64)
    for w in range(WPC):
        blk = w * NC + c
        if blk >= NBLKS:
            continue
        g0 = blk * B
        n = min(B, N - g0)
        out[w * B: w * B + n] = np.arange(g0, g0 + n)
    return out


def _peel_schedule(E):
    src = np.asarray(E[0], dtype=np.int64)
    dst = np.asarray(E[1], dtype=np.int64)
    M = src.shape[0]
    active = np.ones(M, dtype=bool)
    iters = []
    for _ in range(ITERS):
        tgt_cnt = np.zeros(N, np.int64)
        np.add.at(tgt_cnt, dst, active.astype(np.int64))
        use = active & (tgt_cnt == 0)[src]
        ui = use.astype(np.int64)
        node_used = np.zeros(N, np.int64)
        np.maximum.at(node_used, src, ui)
        np.maximum.at(node_used, dst, ui)
        index_map = np.cumsum(node_used) - 1
        s_idx = index_map[src[use]]
        t_idx = index_map[dst[use]]
        cnt = np.zeros(N, np.int64)
        np.add.at(cnt, t_idx, 1)
        iters.append((s_idx, t_idx, cnt))
        active = active & ~use
    return iters


def build_schedule(E):
    """Static schedule: identical program structure for all cores, per-core
    index/matrix data (padded to union shapes)."""
    peel = _peel_schedule(E)
    its = []
    for k in range(ITERS):
        s_idx, t_idx, cnt = peel[k]
        it = {}
        # sources -> AllGather plan
        if k == 0:
            pool_pos, P, src_sched = None, 0, None
        else:
            srcs = np.unique(s_idx)
            per_core = [np.sort(srcs[(srcs // B) % NC == c]) for c in range(NC)]
            P = max(1, max(len(x) for x in per_core))
            pool_pos = {}
            swin_cb = set()
            slot_of = [dict() for _ in range(NC)]
            for c in range(NC):
                for slot, g in enumerate(per_core[c]):
                    g = int(g)
                    pool_pos[g] = c * P + slot
                    slot_of[c][g] = slot
                    lr = _local_row(g)
                    swin_cb.add((lr // B, (lr % B) // 128))
            swin_cb = sorted(swin_cb)
            sc_tables = []
            for (w, cb) in swin_cb:
                tab = np.full((NC, 128), OOB, dtype=np.int32)
                for c in range(NC):
                    blk = w * NC + c
                    if blk >= NBLKS:
                        continue
                    g0 = blk * B + cb * 128
                    for p in range(128):
                        s = slot_of[c].get(g0 + p)
                        if s is not None:
                            tab[c, p] = s
                sc_tables.append(tab)
            src_sched = {"swin_cb": swin_cb, "sc_tables": sc_tables}
        it["P"] = P
        it["src"] = src_sched

        # targets -> gather blocks + aggregation matrices
        tc = (t_idx // B) % NC
        tw = (t_idx // B) // NC
        hotwins = sorted(set(tw.tolist()))
        nblk_w = {}
        for w in hotwins:
            mx = 1
            for c in range(NC):
                ne = int(((tw == w) & (tc == c)).sum())
                mx = max(mx, (ne + 127) // 128)
            nblk_w[w] = mx
        it["hotwins"] = hotwins
        it["nblk_w"] = nblk_w
        it["tw_tc"] = (tw, tc, s_idx, t_idx, cnt, pool_pos)
        its.append(it)

    # second pass: order each iteration's hot windows so windows that go
    # cold next iteration are processed (and gathered) first, then pack
    # the gather/aggregation blocks in that order.
    for k, it in enumerate(its):
        nxt_hot = set(its[k + 1]["hotwins"]) if k + 1 < ITERS else set()
        hot_order = ([w for w in it["hotwins"] if w not in nxt_hot] +
                     [w for w in it["hotwins"] if w in nxt_hot])
        tw, tc, s_idx, t_idx, cnt, pool_pos = it.pop("tw_tc")
        nblk_w = it["nblk_w"]
        nblk_total = sum(nblk_w.values())
        gidx = np.zeros((NC, nblk_total, 128), dtype=np.int32)
        smat = np.zeros((NC, nblk_total, 128, B), dtype=np.float32)
        bpos = 0
        blocks_of_w = {}
        for w in hot_order:
            blocks_of_w[w] = (bpos, nblk_w[w])
            for c in range(NC):
                m = (tw == w) & (tc == c)
                ss, tt = s_idx[m], t_idx[m]
                order = np.argsort(tt, kind="stable")
                ss, tt = ss[order], tt[order]
                for e in range(len(ss)):
                    b = bpos + e // 128
                    p = e % 128
                    gidx[c, b, p] = ss[e] if k == 0 else pool_pos[int(ss[e])]
                    smat[c, b, p, int(tt[e]) % B] = 1.0 / cnt[tt[e]]
            bpos += nblk_w[w]
        it["hot_order"] = hot_order
        it["blocks_of_w"] = blocks_of_w
        it["nblk_total"] = nblk_total
        it["gidx"] = gidx
        it["smat"] = smat
    return its


# ----------------------------------------------------------------------------
# bass program
# ----------------------------------------------------------------------------

def build_bass(sched):
    import concourse.bass as bass
    import concourse.bacc as bacc
    import concourse.mybir as mybir
    import concourse.tile as tile

    bf = mybir.dt.bfloat16
    f8 = mybir.dt.float8e4
    f32 = mybir.dt.float32
    i32 = mybir.dt.int32
    AF = mybir.ActivationFunctionType
    Alu = mybir.AluOpType
    DR = mybir.MatmulPerfMode.DoubleRow

    NGB = sum(it["nblk_total"] for it in sched)
    NSB = sum(len(it["src"]["swin_cb"]) for it in sched if it["src"]) or 1

    nc = bacc.Bacc("TRN2", target_bir_lowering=False, debug=False,
                   enable_asserts=True, num_devices=NC)
    VT0 = nc.dram_tensor("VT0", [WPC, 128, 3, B], bf, kind="ExternalInput").ap()
    VT08 = nc.dram_tensor("VT08", [WPC, 128, 3, B], f8,
                          kind="ExternalInput").ap()
    VF = nc.dram_tensor("VF", [N, D], bf, kind="ExternalInput").ap()
    # fp8 r/z weights (x16): [128k, path(ih,hh), m(0..5), kt, 128]
    WC8 = nc.dram_tensor("WC8", [128, 2, 6, 3, 128], f8,
                         kind="ExternalInput").ap()
    # bf16 n-gate weights: [128k, path(ih,hh), j(0..2), kt, 128]
    WCN = nc.dram_tensor("WCN", [128, 2, 3, 3, 128], bf,
                         kind="ExternalInput").ap()
    BCOL = nc.dram_tensor("BCOL", [128, 12], f32, kind="ExternalInput").ap()
    IDN = nc.dram_tensor("IDN", [128, 128], bf, kind="ExternalInput").ap()
    GIDX = nc.dram_tensor("GIDX", [128, NGB], i32, kind="ExternalInput").ap()
    SIDX = nc.dram_tensor("SIDX", [128, NSB], i32, kind="ExternalInput").ap()
    SMAT = nc.dram_tensor("SMAT", [NGB, 128, B], bf, kind="ExternalInput").ap()
    OUT = nc.dram_tensor("OUT", [WPC, 128, 3, B], bf, kind="ExternalOutput").ap()

    QW = int(os.environ.get("KERNEL_QW", "1"))
    QUADS = [list(range(i, min(i + QW, WPC))) for i in range(0, WPC, QW)]
    GBUFS = max(2, (8 - 3) // QW)
    WB = 4 if QW == 1 else 2   # work-pool rotation depth for per-quad tiles

    with tile.TileContext(nc) as tc:
        with tc.tile_pool(name="const", bufs=1) as cp, \
             tc.tile_pool(name="state", bufs=1) as st, \
             tc.tile_pool(name="work", bufs=2) as wk, \
             tc.tile_pool(name="psum", bufs=2, space="PSUM") as ps, \
             tc.tile_pool(name="dram", bufs=1, space="DRAM") as dp:

            # resident constants
            wc8 = cp.tile([128, 2, 6, 3, 128], f8)
            nc.sync.dma_start(out=wc8[:], in_=WC8[:])
            wcn = cp.tile([128, 2, 3, 3, 128], bf)
            nc.sync.dma_start(out=wcn[:], in_=WCN[:])
            bcol = cp.tile([128, 12], f32)
            nc.sync.dma_start(out=bcol[:], in_=BCOL[:])
            idn = cp.tile([128, 128], bf)
            nc.sync.dma_start(out=idn[:], in_=IDN[:])
            gidx = cp.tile([128, NGB], i32)
            nc.sync.dma_start(out=gidx[:], in_=GIDX[:])
            sidx = cp.tile([128, NSB], i32)
            nc.sync.dma_start(out=sidx[:], in_=SIDX[:])

            # state: single-buffered transposed hidden (bf16) + fp8 shadow
            hT = [st.tile([128, len(q), 3, B], bf, tag=f"hq{qi}",
                          name=f"hq{qi}") for qi, q in enumerate(QUADS)]
            h8 = [st.tile([128, len(q), 3, B], f8, tag=f"h8q{qi}",
                          name=f"h8q{qi}") for qi, q in enumerate(QUADS)]
            for qi, q in enumerate(QUADS):
                for wi, w in enumerate(q):
                    nc.sync.dma_start(out=hT[qi][:, wi, :, :], in_=VT0[w])
                    nc.sync.dma_start(out=h8[qi][:, wi, :, :], in_=VT08[w])
            # per-window q tiles (only hot windows get written)
            qs = [st.tile([128, 3, B], bf, tag=f"q{w}", name=f"q{w}")
                  for w in range(WPC)]
            qs8 = [st.tile([128, 3, B], f8, tag=f"q8{w}", name=f"q8{w}")
                   for w in range(WPC)]

            gpos = 0
            spos = 0
            deferred = [None]
            pend = {}
            GPF = max(it["nblk_total"] for it in sched)

            def emit_gathers(k, src_ap_k):
                nonlocal gpos
                tiles = []
                nb_tot = sched[k]["nblk_total"]
                for _ in range(nb_tot):
                    xg = wk.tile([128, D], bf, tag="xg", bufs=GPF, name="xg")
                    nc.gpsimd.indirect_dma_start(
                        out=xg[:], out_offset=None, in_=src_ap_k[:],
                        in_offset=bass.IndirectOffsetOnAxis(
                            ap=gidx[:, gpos:gpos + 1], axis=0))
                    sm = wk.tile([128, B], bf, tag="sm", bufs=GPF, name="sm")
                    nc.sync.dma_start(out=sm[:], in_=SMAT[gpos])
                    tiles.append((xg, sm))
                    gpos += 1
                return tiles

            # prefetch iteration-0 gathers so phase A starts during init DMAs
            pend["next"] = emit_gathers(0, VF)

            for k in range(ITERS):
                it = sched[k]
                src_cbs = {}
                if k + 1 < ITERS:
                    for (w, cb) in sched[k + 1]["src"]["swin_cb"]:
                        src_cbs.setdefault(w, []).append(cb)
                    P1 = sched[k + 1]["P"]
                    agin = dp.tile([P1, D], bf, tag=f"agin{k+1}",
                                   name=f"agin{k+1}")
                    agout = dp.tile([NC * P1, D], bf, tag=f"agout{k+1}",
                                    name=f"agout{k+1}", addr_space="Shared")

                # phase A helper: aggregate one hot window's gathered blocks
                # (emitted just before that window's stage1 so the PE queue
                # never head-of-line-blocks on not-yet-arrived gathers)
                def window_agg(w, it=it):
                    pending = pend["next"]
                    bpos, nb = it["blocks_of_w"][w]
                    qp = ps.tile([128, 3, 512], f32, tag="qp",
                                 space="PSUM", name="qp", bufs=1)
                    for bi in range(nb):
                        xg, sm = pending[bpos + bi]
                        for kt in range(3):
                            nc.tensor.matmul(
                                qp[:, kt, :B],
                                lhsT=xg[:, kt * 128:(kt + 1) * 128],
                                rhs=sm[:],
                                start=(bi == 0), stop=(bi == nb - 1))
                    nc.vector.tensor_copy(qs[w][:], qp[:, 0:3, :B])
                    nc.scalar.activation(qs8[w][:], qp[:, 0:3, :B],
                                         AF.Identity)

                # phase B, one quad at a time; each quad's tanh/h' tail is
                # deferred until after the next quad's matmul stage so the
                # ACT/DVE FIFOs never head-of-line-block the PE.  Cold
                # quads run first (covering the previous AllGather), hot
                # quads follow in hot_order (next-iteration-cold first so
                # their tails land early for the next boundary's cover).
                hot_q = set(qi for qi, q in enumerate(QUADS)
                            if any(w in it["hotwins"] for w in q))
                def srcy(q):
                    return any(w in src_cbs for w in q)
                src_first = [qi for qi in range(len(QUADS))
                             if qi not in hot_q and srcy(QUADS[qi])]
                plain_cold = [qi for qi in range(len(QUADS))
                              if qi not in hot_q and not srcy(QUADS[qi])]
                pos = {w: i for i, w in enumerate(it["hot_order"])}
                hot_sorted = sorted(hot_q,
                                    key=lambda qi: min(pos.get(w, 99)
                                                       for w in QUADS[qi]))
                qorder = src_first + plain_cold + hot_sorted
                src_quads = [qi for qi in qorder if srcy(QUADS[qi])]
                last_src_qi = src_quads[-1] if src_quads else None
                agg_done = set()
                for qi in qorder:
                    q = QUADS[qi]
                    # if the deferred tail writes this quad's h, it must land
                    # before this quad's stage1 reads it
                    if deferred[0] is not None and deferred[0][1] == qi:
                        deferred[0][0]()
                        deferred[0] = None
                    for w in q:
                        if w in it["hotwins"] and w not in agg_done:
                            window_agg(w)
                            agg_done.add(w)
                    nq = len(q)
                    hotm = [wi for wi, w in enumerate(q) if w in it["hotwins"]]
                    hq = hT[qi]
                    h8q = h8[qi]

                    def rz_group(m, with_q):
                        # fp8 r/z gate block m: DoubleRow(kt0,kt1) + plain kt2
                        gp = ps.tile([128, nq, 512], f32, tag="gg",
                                     space="PSUM", name="gp", bufs=GBUFS)
                        for wi in range(nq):
                            qhot = with_q and wi in hotm
                            nc.tensor.matmul(
                                gp[:, wi, :B],
                                lhsT=wc8[:, 1, m, 0:2, :],
                                rhs=h8q[:, wi, 0:2, :],
                                start=True, stop=False, perf_mode=DR)
                            nc.tensor.matmul(
                                gp[:, wi, :B],
                                lhsT=wc8[:, 1, m, 2, :],
                                rhs=h8q[:, wi, 2, :],
                                start=False, stop=not qhot)
                            if qhot:
                                nc.tensor.matmul(
                                    gp[:, wi, :B],
                                    lhsT=wc8[:, 0, m, 0:2, :],
                                    rhs=qs8[q[wi]][:, 0:2, :],
                                    start=False, stop=False, perf_mode=DR)
                                nc.tensor.matmul(
                                    gp[:, wi, :B],
                                    lhsT=wc8[:, 0, m, 2, :],
                                    rhs=qs8[q[wi]][:, 2, :],
                                    start=False, stop=True)
                        return gp

                    def n_group(j, path, rhs_of):
                        # bf16 n-gate block j for one path (0=ih/q, 1=hh/h)
                        gp = ps.tile([128, nq, 512], f32, tag="gg",
                                     space="PSUM", name="gp", bufs=GBUFS)
                        for wi in rhs_of:
                            for kt in range(3):
                                nc.tensor.matmul(
                                    gp[:, wi, :B],
                                    lhsT=wcn[:, path, j, kt, :],
                                    rhs=rhs_of[wi][:, kt, :],
                                    start=(kt == 0), stop=(kt == 2))
                        return gp

                    anyhot = bool(hotm)
                    r_sb = wk.tile([128, nq, 3, B], bf, tag="r", bufs=WB,
                                   name="r_sb")
                    z_sb = wk.tile([128, nq, 3, B], bf, tag="z", bufs=WB,
                                   name="z_sb")
                    for j in range(3):
                        rp = rz_group(j, anyhot)
                        nc.scalar.activation(r_sb[:, :, j, :], rp[:, :, :B],
                                             AF.Sigmoid, bias=bcol[:, j:j + 1],
                                             scale=1.0 / W8SCALE)
                    for j in range(3):
                        zp = rz_group(3 + j, anyhot)
                        nc.scalar.activation(z_sb[:, :, j, :], zp[:, :, :B],
                                             AF.Sigmoid,
                                             bias=bcol[:, 3 + j:4 + j],
                                             scale=1.0 / W8SCALE)
                    t2 = wk.tile([128, nq, 3, B], bf, tag="t2", bufs=WB,
                                 name="t2")
                    h_rhs = {wi: hq[:, wi] for wi in range(nq)}
                    q_rhs = {wi: qs[q[wi]] for wi in hotm}
                    t1 = (wk.tile([128, nq, 3, B], bf, tag="t1", bufs=WB,
                                  name="t1") if anyhot else None)
                    for j in range(3):
                        hp = n_group(j, 1, h_rhs)
                        if anyhot:
                            nc.vector.scalar_tensor_tensor(
                                out=t1[:, :, j, :], in0=hp[:, :, :B],
                                scalar=bcol[:, 6 + j:7 + j],
                                in1=r_sb[:, :, j, :],
                                op0=Alu.add, op1=Alu.mult)
                            ip = n_group(j, 0, q_rhs)
                            # hot: t2 = (gi_n + b_ih_n) + t1, so tanh needs
                            # no bias; cold: t2 = t1, bias rides the tanh
                            for wi in range(nq):
                                if wi in hotm:
                                    nc.vector.scalar_tensor_tensor(
                                        out=t2[:, wi, j, :],
                                        in0=ip[:, wi, :B],
                                        scalar=bcol[:, 9 + j:10 + j],
                                        in1=t1[:, wi, j, :],
                                        op0=Alu.add, op1=Alu.add)
                                else:
                                    nc.vector.tensor_copy(
                                        t2[:, wi, j, :], t1[:, wi, j, :])
                        else:
                            nc.vector.scalar_tensor_tensor(
                                out=t2[:, :, j, :], in0=hp[:, :, :B],
                                scalar=bcol[:, 6 + j:7 + j],
                                in1=r_sb[:, :, j, :],
                                op0=Alu.add, op1=Alu.mult)

                    def tail(q=q, nq=nq, qi=qi, hq=hq, h8q=h8q, t2=t2,
                             z_sb=z_sb, src_cbs=src_cbs, k=k, hotm=hotm,
                             fire=(qi == last_src_qi),
                             agin=agin if k + 1 < ITERS else None,
                             agout=agout if k + 1 < ITERS else None,
                             P1=P1 if k + 1 < ITERS else None):
                        nonlocal spos
                        n_sb = wk.tile([128, nq, 3, B], bf, tag="n", bufs=WB,
                                       name="n_sb")
                        d_sb = wk.tile([128, nq, 3, B], bf, tag="d", bufs=WB,
                                       name="d_sb")
                        e_sb = wk.tile([128, nq, 3, B], bf, tag="e", bufs=WB,
                                       name="e_sb")
                        has_src = any(w in src_cbs for w in q)
                        per_win = has_src or bool(hotm)
                        wins = [(wi, slice(None)) for wi in range(nq)] \
                            if per_win else [(slice(None), slice(None))]
                        my_src = [wi for wi in range(nq) if q[wi] in src_cbs]
                        fire_w = q[max(my_src)] if (fire and my_src) else None
                        for wsel, _ in wins:
                            if per_win and wsel in hotm:
                                # bias already folded into t2 by the stt
                                nc.scalar.activation(
                                    n_sb[:, wsel], t2[:, wsel], AF.Tanh)
                            else:
                                for j in range(3):
                                    nc.scalar.activation(
                                        n_sb[:, wsel, j, :], t2[:, wsel, j, :],
                                        AF.Tanh, bias=bcol[:, 9 + j:10 + j])
                            nc.vector.tensor_sub(out=d_sb[:, wsel],
                                                 in0=hq[:, wsel],
                                                 in1=n_sb[:, wsel])
                            nc.vector.tensor_mul(out=e_sb[:, wsel],
                                                 in0=z_sb[:, wsel],
                                                 in1=d_sb[:, wsel])
                            nc.vector.tensor_add(out=hq[:, wsel],
                                                 in0=n_sb[:, wsel],
                                                 in1=e_sb[:, wsel])
                            nc.vector.tensor_copy(h8q[:, wsel], hq[:, wsel])
                            if has_src:
                                wi = wsel
                                w = q[wi]
                                for cb in src_cbs.get(w, []):
                                    tp = ps.tile([128, B], bf, tag="gg",
                                                 space="PSUM", name="tp",
                                                 bufs=GBUFS)
                                    for kt in range(3):
                                        nc.tensor.transpose(
                                            tp[:, kt * 128:(kt + 1) * 128],
                                            hq[:, wi, kt,
                                               cb * 128:(cb + 1) * 128],
                                            idn[:])
                                    rm = wk.tile([128, D], bf, tag="rm")
                                    nc.vector.tensor_copy(rm[:], tp[:])
                                    nc.gpsimd.indirect_dma_start(
                                        out=agin[:],
                                        out_offset=bass.IndirectOffsetOnAxis(
                                            ap=sidx[:, spos:spos + 1], axis=0),
                                        in_=rm[:], in_offset=None,
                                        bounds_check=P1 - 1, oob_is_err=False)
                                    spos += 1
                                if w == fire_w:
                                    nc.gpsimd.collective_compute(
                                        "AllGather", Alu.bypass,
                                        replica_groups=[list(range(NC))],
                                        ins=[agin[:].opt()],
                                        outs=[agout[:].opt()])
                                    pend["next"] = emit_gathers(k + 1, agout)

                    if deferred[0] is not None:
                        deferred[0][0]()
                    deferred[0] = (tail, qi)

            if deferred[0] is not None:
                deferred[0][0]()
                deferred[0] = None
            for qi, q in enumerate(QUADS):
                for wi, w in enumerate(q):
                    nc.sync.dma_start(out=OUT[w], in_=hT[qi][:, wi, :, :])
    nc.compile()
    return nc


# ----------------------------------------------------------------------------
# host packing + entry point
# ----------------------------------------------------------------------------

def pack_inputs(sched, c, V, conv_weight, w_ih, w_hh, b_ih, b_hh):
    V = np.asarray(V, dtype=np.float32)
    Wcat = np.concatenate([np.asarray(conv_weight) @ np.asarray(w_ih).T,
                           np.asarray(w_hh).T], axis=0).astype(np.float32)
    b_ih = np.asarray(b_ih, dtype=np.float32)
    b_hh = np.asarray(b_hh, dtype=np.float32)

    grows = _global_rows_of(c)
    hl = np.zeros((LROWS, D), dtype=np.float32)
    valid = grows >= 0
    hl[valid] = V[grows[valid]]
    # VT0[w, p, kt, j] = h[w*B + j, kt*128 + p]
    vt0f = np.ascontiguousarray(
        hl.reshape(WPC, B, 3, 128).transpose(0, 3, 2, 1))
    vt0 = vt0f.astype(BF16)
    vt08 = vt0f.astype(BF16).astype(FP8)
    # WC8[p, path, m, kt, :]: path 0 -> Wcat rows (ih), 1 -> rows 384+ (hh)
    # m = output block 0..5 (r then z), x16 scale
    wc8 = np.zeros((128, 2, 6, 3, 128), dtype=np.float32)
    wcn = np.zeros((128, 2, 3, 3, 128), dtype=np.float32)
    for path in range(2):
        for kt in range(3):
            krow = (path * 3 + kt) * 128
            for m in range(6):
                wc8[:, path, m, kt, :] = (
                    Wcat[krow:krow + 128, m * 128:(m + 1) * 128] * W8SCALE)
            for j in range(3):
                mm = 6 + j
                wcn[:, path, j, kt, :] = \
                    Wcat[krow:krow + 128, mm * 128:(mm + 1) * 128]
    bsum = b_ih + b_hh
    bl = np.zeros((12, 128), dtype=np.float32)
    for m in range(6):
        bl[m] = bsum[m * 128:(m + 1) * 128]
    for j in range(3):
        bl[6 + j] = b_hh[768 + j * 128: 768 + (j + 1) * 128]
        bl[9 + j] = b_ih[768 + j * 128: 768 + (j + 1) * 128]
    bc = np.ascontiguousarray(bl.T)  # [128, 12]

    gidx = np.concatenate([it["gidx"][c] for it in sched], axis=0)  # [NGB,128]
    smat = np.concatenate([it["smat"][c] for it in sched], axis=0)  # [NGB,128,B]
    sc = [tab[c] for it in sched if it["src"] for tab in it["src"]["sc_tables"]]
    sidx = (np.stack(sc, axis=0) if sc else np.zeros((1, 128), np.int32))

    return {
        "VT0": vt0,
        "VT08": vt08,
        "VF": V.astype(BF16),
        "WC8": wc8.astype(FP8),
        "WCN": wcn.astype(BF16),
        "BCOL": bc.astype(np.float32),
        "IDN": np.eye(128, dtype=np.float32).astype(BF16),
        "GIDX": np.ascontiguousarray(gidx.T).astype(np.int32),
        "SIDX": np.ascontiguousarray(sidx.T).astype(np.int32),
        "SMAT": smat.astype(BF16),
    }


def unpack_output(results):
    out = np.zeros((N, D), dtype=np.float32)
    for c in range(NC):
        o = np.asarray(results[c]["OUT"], dtype=np.float32)  # [WPC,128,3,B]
        hl = o.transpose(0, 3, 2, 1).reshape(LROWS, D)
        grows = _global_rows_of(c)
        valid = grows >= 0
        out[grows[valid]] = hl[valid]
    return out


_CACHE = {}


def _install_profile_hook():
    """The agent image lacks ``antenv.axon_hooks``; shim it so
    run_bass_kernel_spmd(trace=True) can capture NTFF profiles."""
    import types
    try:
        from antenv.axon_hooks import get_axon_ntff_profile_hook  # noqa: F401
        return True
    except ImportError:
        pass
    try:
        import antenv
        from trn_agent_boot.trn_boot import _ntff_profile_via_ctypes
        hook = _ntff_profile_via_ctypes("/opt/axon/libaxon_pjrt.so")
        mod = types.ModuleType("antenv.axon_hooks")
        mod._hook = hook
        mod.set_axon_ntff_profile_hook = lambda h: setattr(mod, "_hook", h)
        mod.get_axon_ntff_profile_hook = lambda: mod._hook
        sys.modules["antenv.axon_hooks"] = mod
        antenv.axon_hooks = mod
        return hook is not None
    except Exception:
        return False


def kernel(V, E, conv_weight, w_ih, w_hh, b_ih, b_hh, _want_results=False):
    from concourse import bass_utils
    E_np = np.asarray(E)
    sched = build_schedule(E_np)
    key = tuple((it["nblk_total"], it["P"], tuple(it["hotwins"]),
                 tuple(it["src"]["swin_cb"]) if it["src"] else ())
                for it in sched)
    if key not in _CACHE:
        _CACHE[key] = build_bass(sched)
    nc = _CACHE[key]
    in_maps = [pack_inputs(sched, c, V, conv_weight, w_ih, w_hh, b_ih, b_hh)
               for c in range(NC)]
    trace = os.environ.get("KERNEL_TRACE", "0") == "1"
    if trace:
        trace = _install_profile_hook()
        # artifact upload to the fish bucket is unavailable here; stub it
        bass_utils.upload_artifacts = lambda tmpdir: "local://" + str(tmpdir)
    res = bass_utils.run_bass_kernel_spmd(
        nc, in_maps, core_ids=list(range(NC)), trace=trace,
        tmpdir=os.environ.get("KERNEL_TMPDIR"))
    out = unpack_output(res.results).astype(np.float32)
    if _want_results:
        return out, res
    return out

